# revision 1
# baseline (speedup 1.0000x reference)
# Cross-attention kernel for Trainium2 (Bass/Tile), 8-core data-parallel.
#
# Reference computation (per batch element, B=8 -> one batch element per core):
#   q = x1 @ Wq.T + bq ; k = x2 @ Wk.T + bk ; v = x3 @ Wv.T + bv
#   out = softmax(q @ k.T) @ v          (no 1/sqrt(d) scale)
#
# Precision strategy (validated numerically against the fp32 reference,
# absmax rel err ~4e-3):
#   - q,k projections and q@k.T run as 3-pass bf16 hi/lo split matmuls
#     (hi = bf16(x), lo = bf16(x - hi); x@y ~= xh@yh + xh@yl + xl@yh),
#     accumulated in fp32 PSUM. Effective precision ~fp32 for the scores,
#     which matters because the unscaled scores have std ~46 and the softmax
#     is extremely sharp.
#   - v projection and attn@v run in plain bf16 (error contribution ~2e-3).
#   - softmax itself is fp32 (row max subtraction on-chip, exp on ScalarE,
#     normalization deferred to the output).
#
# Layout strategy per core (S=2048, C=1024, P=128):
#   - qT, kT computed directly transposed ([d, s], d on partitions) so the
#     score matmul contracts over d. All transposes (W, x, p) run on the DMA
#     xbar (2-byte transpose mode, SP HWDGE queue) -- they never touch the PE
#     or vector engines.
#   - v computed in natural [s, c] layout (stationary operand for attn@v).
#   - kT(hi/lo) and v stay resident in SBUF; qT(hi/lo) spills to a DRAM
#     scratch and streams back per 128-row query tile (ACT HWDGE queue, so
#     plain copies and xbar transposes live on different queues).
#   - p = exp(s - rowmax) transposed per sq-tile in one xbar DMA; the row sum
#     rides along via the activation accumulator, output normalized at the end.

from contextlib import ExitStack

import numpy as np

import concourse.bass as bass
import concourse.mybir as mybir
import concourse.tile as tile
from concourse import bacc
from concourse.bass_utils import run_bass_kernel_spmd

F32 = mybir.dt.float32
BF16 = mybir.dt.bfloat16
ADD = mybir.AluOpType.add
SUB = mybir.AluOpType.subtract
AX = mybir.AxisListType.X
EXP = mybir.ActivationFunctionType.Exp

B, S, C = 8, 2048, 1024
P = 128
NT_S = S // P  # 16 s-tiles
NT_C = C // P  # 8 c/d-tiles
CH = 512  # free-dim chunk (one fp32 PSUM bank)
NCH_S = S // CH  # 4
NCH_C = C // CH  # 2


def _emit(tc):
    nc = tc.nc

    x1 = nc.dram_tensor("x1", [S, C], F32, kind="ExternalInput").ap()
    x2 = nc.dram_tensor("x2", [S, C], F32, kind="ExternalInput").ap()
    x3 = nc.dram_tensor("x3", [S, C], F32, kind="ExternalInput").ap()
    Wq = nc.dram_tensor("Wq", [C, C], F32, kind="ExternalInput").ap()
    Wk = nc.dram_tensor("Wk", [C, C], F32, kind="ExternalInput").ap()
    Wv = nc.dram_tensor("Wv", [C, C], F32, kind="ExternalInput").ap()
    bq = nc.dram_tensor("bq", [C], F32, kind="ExternalInput").ap()
    bk = nc.dram_tensor("bk", [C], F32, kind="ExternalInput").ap()
    bv = nc.dram_tensor("bv", [C], F32, kind="ExternalInput").ap()
    out = nc.dram_tensor("out", [S, C], F32, kind="ExternalOutput").ap()

    es = ExitStack()
    with es:
        const = es.enter_context(tc.tile_pool(name="const", bufs=1))
        dram = es.enter_context(tc.tile_pool(name="dram", bufs=1, space="DRAM"))

        # biases: bq/bk as per-d-tile columns [128, 8]; bv broadcast [128, C]
        bq_sb = const.tile([P, NT_C], F32, tag="bq")
        nc.scalar.dma_start(out=bq_sb, in_=bq.rearrange("(t p) -> p t", p=P))
        bk_sb = const.tile([P, NT_C], F32, tag="bk")
        nc.scalar.dma_start(out=bk_sb, in_=bk.rearrange("(t p) -> p t", p=P))
        bv_sb = const.tile([P, C], F32, tag="bv")
        bv_bcast = bass.AP(tensor=bv.tensor, offset=bv.offset, ap=[[0, P], [1, C]])
        nc.scalar.dma_start(out=bv_sb, in_=bv_bcast)

        # DRAM scratch for spilled qT (hi/lo)
        qTh_d = dram.tile([NT_C, P, S], BF16, tag="qThd", name="qThd")
        qTl_d = dram.tile([NT_C, P, S], BF16, tag="qTld", name="qTld")

        def prep_w(W, wpool, split):
            """Load W [C,C] (rows d, cols c); produce W^T as one 3D tile
            [128c, NT_C(ct), C(d)] bf16 hi (and lo) via xbar transposes."""
            WhT = wpool.tile([P, NT_C, C], BF16, tag="WhT", name="WhT")
            WlT = wpool.tile([P, NT_C, C], BF16, tag="WlT", name="WlT") if split else None
            with tc.tile_pool(name="wstage", bufs=2) as ws:
                for dt in range(NT_C):
                    wnat = ws.tile([P, C], F32, tag="wnat", name="wnat")
                    weng = nc.gpsimd if dt % 2 == 0 else nc.scalar
                    weng.dma_start(out=wnat, in_=W[dt * P : (dt + 1) * P, :])
                    wh = ws.tile([P, C], BF16, tag="wh", name="wh")
                    nc.vector.tensor_copy(out=wh, in_=wnat)
                    nc.sync.dma_start(
                        out=WhT[:, :, dt * P : (dt + 1) * P], in_=wh, transpose=True
                    )
                    if split:
                        wl = ws.tile([P, C], BF16, tag="wl", name="wl")
                        nc.vector.tensor_tensor(out=wl, in0=wnat, in1=wh, op=SUB)
                        nc.sync.dma_start(
                            out=WlT[:, :, dt * P : (dt + 1) * P], in_=wl, transpose=True
                        )
            return WhT, WlT

        def prep_xT_chunk(x, s0, split, xs_pool, xt_pool):
            """Load x[s0:s0+CH, :] one s-tile at a time, split hi/lo, and xbar-
            transpose into [128c, NT_C(ct), CH(s)] bf16 tiles (hi, lo)."""
            nj = CH // P  # 4 s-tiles per chunk
            xhT = xt_pool.tile([P, NT_C, CH], BF16, tag="xhT", name="xhT")
            xlT = (
                xt_pool.tile([P, NT_C, CH], BF16, tag="xlT", name="xlT")
                if split
                else None
            )
            for j in range(nj):
                r0 = s0 + j * P
                xs = xs_pool.tile([P, C], F32, tag="xload", name="xload")
                xeng = nc.gpsimd if j % 2 == 0 else nc.scalar
                xeng.dma_start(out=xs, in_=x[r0 : r0 + P, :])
                xh = xs_pool.tile([P, C], BF16, tag="xh", name="xh")
                nc.vector.tensor_copy(out=xh, in_=xs)
                nc.sync.dma_start(
                    out=xhT[:, :, j * P : (j + 1) * P], in_=xh, transpose=True
                )
                if split:
                    xl = xs_pool.tile([P, C], BF16, tag="xl", name="xl")
                    nc.vector.tensor_tensor(out=xl, in0=xs, in1=xh, op=SUB)
                    nc.sync.dma_start(
                        out=xlT[:, :, j * P : (j + 1) * P], in_=xl, transpose=True
                    )
            return xhT, xlT

        def split_proj_mms(ps, xhT, xlT, WhT, WlT, dt):
            """Emit the 24 matmuls of a 3-pass split projection into psum ps."""
            n_mm = NT_C * 3
            i = 0
            for ct in range(NT_C):
                lw_h = WhT[:, ct, dt * P : (dt + 1) * P]
                nc.tensor.matmul(
                    ps, lw_h, xhT[:, ct, :], start=(i == 0), stop=(i == n_mm - 1)
                )
                i += 1
                nc.tensor.matmul(
                    ps, lw_h, xlT[:, ct, :], start=False, stop=(i == n_mm - 1)
                )
                i += 1
                lw_l = WlT[:, ct, dt * P : (dt + 1) * P]
                nc.tensor.matmul(
                    ps, lw_l, xhT[:, ct, :], start=False, stop=(i == n_mm - 1)
                )
                i += 1

        # ---------------- Phase Q: project qT (hi/lo) -> DRAM scratch --------
        with tc.tile_pool(name="wq", bufs=1) as wq_pool:
            WqhT, WqlT = prep_w(Wq, wq_pool, split=True)
            with (
                tc.tile_pool(name="qxs", bufs=2) as qxs,
                tc.tile_pool(name="qxt", bufs=2) as qxt,
                tc.tile_pool(name="qmmps", bufs=2, space="PSUM") as qmmps,
                tc.tile_pool(name="qst", bufs=3) as qst,
            ):
                nxt = prep_xT_chunk(x1, 0, True, qxs, qxt)
                for ich in range(NCH_S):
                    s0 = ich * CH
                    xhT, xlT = nxt
                    if ich + 1 < NCH_S:
                        nxt = prep_xT_chunk(x1, (ich + 1) * CH, True, qxs, qxt)
                    for dt in range(NT_C):
                        ps = qmmps.tile([P, CH], F32, tag="projps", name="projps")
                        split_proj_mms(ps, xhT, xlT, WqhT, WqlT, dt)
                        t = qst.tile([P, CH], F32, tag="projt", name="projt")
                        nc.vector.tensor_scalar_add(
                            out=t, in0=ps, scalar1=bq_sb[:, dt : dt + 1]
                        )
                        h = qst.tile([P, CH], BF16, tag="projh", name="projh")
                        nc.scalar.copy(out=h, in_=t)
                        l = qst.tile([P, CH], BF16, tag="projl", name="projl")
                        nc.vector.tensor_tensor(out=l, in0=t, in1=h, op=SUB)
                        nc.scalar.dma_start(out=qTh_d[dt, :, s0 : s0 + CH], in_=h)
                        nc.scalar.dma_start(out=qTl_d[dt, :, s0 : s0 + CH], in_=l)

        # ---------------- Phase K: project kT (hi/lo) -> resident SBUF -------
        res_k = es.enter_context(tc.tile_pool(name="resk", bufs=1))
        kTh = [
            res_k.tile([P, S], BF16, tag=f"kTh{i}", name=f"kTh{i}")
            for i in range(NT_C)
        ]
        kTl = [
            res_k.tile([P, S], BF16, tag=f"kTl{i}", name=f"kTl{i}")
            for i in range(NT_C)
        ]
        with tc.tile_pool(name="wk", bufs=1) as wk_pool:
            WkhT, WklT = prep_w(Wk, wk_pool, split=True)
            with (
                tc.tile_pool(name="kxs", bufs=2) as kxs,
                tc.tile_pool(name="kxt", bufs=2) as kxt,
                tc.tile_pool(name="kmmps", bufs=2, space="PSUM") as kmmps,
                tc.tile_pool(name="kst", bufs=3) as kst,
            ):
                nxt = prep_xT_chunk(x2, 0, True, kxs, kxt)
                for ich in range(NCH_S):
                    s0 = ich * CH
                    xhT, xlT = nxt
                    if ich + 1 < NCH_S:
                        nxt = prep_xT_chunk(x2, (ich + 1) * CH, True, kxs, kxt)
                    for dt in range(NT_C):
                        ps = kmmps.tile([P, CH], F32, tag="projps", name="kprojps")
                        split_proj_mms(ps, xhT, xlT, WkhT, WklT, dt)
                        t = kst.tile([P, CH], F32, tag="projt", name="kprojt")
                        nc.vector.tensor_scalar_add(
                            out=t, in0=ps, scalar1=bk_sb[:, dt : dt + 1]
                        )
                        h_sl = kTh[dt][:, s0 : s0 + CH]
                        nc.scalar.copy(out=h_sl, in_=t)
                        nc.vector.tensor_tensor(
                            out=kTl[dt][:, s0 : s0 + CH], in0=t, in1=h_sl, op=SUB
                        )

        # ---------------- Phase V: project v (natural [s, c]) -> resident ----
        res_v = es.enter_context(tc.tile_pool(name="resv", bufs=1))
        v_res = [
            res_v.tile([P, C], BF16, tag=f"v{i}", name=f"v{i}") for i in range(NT_S)
        ]
        with tc.tile_pool(name="wv", bufs=1) as wv_pool:
            WvhT, _ = prep_w(Wv, wv_pool, split=False)
            with (
                tc.tile_pool(name="vxs", bufs=2) as vxs,
                tc.tile_pool(name="vxt", bufs=2) as vxt,
                tc.tile_pool(name="vmmps", bufs=2, space="PSUM") as vmmps,
            ):
                nxt3 = prep_xT_chunk(x3, 0, False, vxs, vxt)
                for ich in range(NCH_S):
                    s0 = ich * CH
                    x3hT, _ = nxt3
                    if ich + 1 < NCH_S:
                        nxt3 = prep_xT_chunk(x3, (ich + 1) * CH, False, vxs, vxt)
                    for j in range(CH // P):  # s-tile within chunk
                        st = ich * (CH // P) + j
                        for cch in range(NCH_C):
                            ps = vmmps.tile([P, CH], F32, tag="vps", name="vps")
                            for ct in range(NT_C):
                                nc.tensor.matmul(
                                    ps,
                                    x3hT[:, ct, j * P : (j + 1) * P],
                                    WvhT[:, ct, cch * CH : (cch + 1) * CH],
                                    start=(ct == 0),
                                    stop=(ct == NT_C - 1),
                                )
                            nc.vector.tensor_tensor(
                                out=v_res[st][:, cch * CH : (cch + 1) * CH],
                                in0=ps,
                                in1=bv_sb[:, cch * CH : (cch + 1) * CH],
                                op=ADD,
                            )

        # ---------------- Attention ------------------------------------------
        with (
            tc.tile_pool(name="qstream", bufs=2) as qstream,
            tc.tile_pool(name="spsum", bufs=6, space="PSUM") as spsum,
            tc.tile_pool(name="opsum", bufs=2, space="PSUM") as opsum,
            tc.tile_pool(name="attn", bufs=2) as attn,
            tc.tile_pool(name="stats", bufs=4) as stats,
        ):
            for sq in range(NT_S):
                qh_t = qstream.tile([P, NT_C, P], BF16, tag="qh", name="qh")
                nc.scalar.dma_start(
                    out=qh_t,
                    in_=qTh_d[:, :, sq * P : (sq + 1) * P].rearrange("t p s -> p t s"),
                )
                ql_t = qstream.tile([P, NT_C, P], BF16, tag="ql", name="ql")
                nc.scalar.dma_start(
                    out=ql_t,
                    in_=qTl_d[:, :, sq * P : (sq + 1) * P].rearrange("t p s -> p t s"),
                )

                # scores: s[sq-tile, :] accumulated over d in 4 chunk banks
                ps_s = [
                    spsum.tile([P, CH], F32, tag="s", name=f"s{c}")
                    for c in range(NCH_S)
                ]
                cnt = [0] * NCH_S
                n_per = NT_C * 3
                for dt in range(NT_C):
                    qh_sl = qh_t[:, dt, :]
                    ql_sl = ql_t[:, dt, :]
                    for c in range(NCH_S):
                        nc.tensor.matmul(
                            ps_s[c],
                            qh_sl,
                            kTh[dt][:, c * CH : (c + 1) * CH],
                            start=(cnt[c] == 0),
                            stop=(cnt[c] == n_per - 1),
                        )
                        cnt[c] += 1
                    for c in range(NCH_S):
                        nc.tensor.matmul(
                            ps_s[c],
                            qh_sl,
                            kTl[dt][:, c * CH : (c + 1) * CH],
                            start=False,
                            stop=(cnt[c] == n_per - 1),
                        )
                        cnt[c] += 1
                    for c in range(NCH_S):
                        nc.tensor.matmul(
                            ps_s[c],
                            ql_sl,
                            kTh[dt][:, c * CH : (c + 1) * CH],
                            start=False,
                            stop=(cnt[c] == n_per - 1),
                        )
                        cnt[c] += 1

                # softmax (fp32, row-wise over the free dim)
                mx = stats.tile([P, NCH_S], F32, tag="mx", name="mx")
                for c in range(NCH_S):
                    nc.vector.reduce_max(out=mx[:, c : c + 1], in_=ps_s[c], axis=AX)
                negmax = stats.tile([P, 1], F32, tag="negmax", name="negmax")
                nc.vector.reduce_max(out=negmax, in_=mx, axis=AX, negate=True)

                p_sb = attn.tile([P, S], BF16, tag="p", name="p")
                sums = stats.tile([P, NCH_S], F32, tag="sums", name="sums")
                for c in range(NCH_S):
                    nc.scalar.activation(
                        out=p_sb[:, c * CH : (c + 1) * CH],
                        in_=ps_s[c],
                        func=EXP,
                        bias=negmax,
                        scale=1.0,
                        accum_out=sums[:, c : c + 1],
                    )
                rs = stats.tile([P, 1], F32, tag="rs", name="rs")
                nc.vector.reduce_sum(out=rs, in_=sums, axis=AX)
                rinv = stats.tile([P, 1], F32, tag="rinv", name="rinv")
                nc.vector.reciprocal(out=rinv, in_=rs)

                # transpose p for attn @ v: one xbar DMA per sq-tile
                pT = attn.tile([P, NT_S, P], BF16, tag="pT", name="pT")
                nc.sync.dma_start(out=pT, in_=p_sb, transpose=True)

                # attn @ v, accumulate over sk tiles; normalize; store
                ps_o = [
                    opsum.tile([P, CH], F32, tag="o", name=f"o{c}")
                    for c in range(NCH_C)
                ]
                for skt in range(NT_S):
                    for cch in range(NCH_C):
                        nc.tensor.matmul(
                            ps_o[cch],
                            pT[:, skt, :],
                            v_res[skt][:, cch * CH : (cch + 1) * CH],
                            start=(skt == 0),
                            stop=(skt == NT_S - 1),
                        )
                o_sb = attn.tile([P, C], F32, tag="osb", name="osb")
                for cch in range(NCH_C):
                    nc.vector.tensor_scalar_mul(
                        out=o_sb[:, cch * CH : (cch + 1) * CH],
                        in0=ps_o[cch],
                        scalar1=rinv,
                    )
                nc.scalar.dma_start(out=out[sq * P : (sq + 1) * P, :], in_=o_sb)


_BUILT = {}


def _build():
    if "nc" not in _BUILT:
        nc = bacc.Bacc(
            "TRN2",
            target_bir_lowering=False,
            debug=False,
            num_devices=B,
        )
        with tile.TileContext(nc) as tc:
            _emit(tc)
        nc.compile()
        _BUILT["nc"] = nc
    return _BUILT["nc"]


def kernel_with_results(trace=False, **inputs):
    nc = _build()
    in_maps = []
    for i in range(B):
        in_maps.append(
            {
                "x1": np.ascontiguousarray(inputs["x1"][i], dtype=np.float32),
                "x2": np.ascontiguousarray(inputs["x2"][i], dtype=np.float32),
                "x3": np.ascontiguousarray(inputs["x3"][i], dtype=np.float32),
                "Wq": np.ascontiguousarray(inputs["Wq"], dtype=np.float32),
                "Wk": np.ascontiguousarray(inputs["Wk"], dtype=np.float32),
                "Wv": np.ascontiguousarray(inputs["Wv"], dtype=np.float32),
                "bq": np.ascontiguousarray(inputs["bq"], dtype=np.float32),
                "bk": np.ascontiguousarray(inputs["bk"], dtype=np.float32),
                "bv": np.ascontiguousarray(inputs["bv"], dtype=np.float32),
            }
        )
    res = run_bass_kernel_spmd(nc, in_maps, core_ids=list(range(B)), trace=trace)
    outs = np.stack([r["out"] for r in res.results], axis=0).astype(np.float32)
    return outs, res


def kernel(**inputs):
    outs, _ = kernel_with_results(trace=False, **inputs)
    return outs



# revision 2
# speedup vs baseline: 2.3185x; 2.3185x over previous
# Cross-attention kernel for Trainium2 (Bass/Tile), 8-core data-parallel.
#
# Reference computation (per batch element, B=8 -> one batch element per core):
#   q = x1 @ Wq.T + bq ; k = x2 @ Wk.T + bk ; v = x3 @ Wv.T + bv
#   out = softmax(q @ k.T) @ v          (no 1/sqrt(d) scale)
#
# Precision strategy (validated on HW):
#   - q/k projections and q@k.T run as single-pass fp32r matmuls. fp32r is
#     the PE's hw bf16 hi/lo pair decomposition of fp32: measured rel err
#     ~1.3e-4 over a K=1024 contraction, and it streams at bf16 speed when
#     the moving free dim is >= 256. This replaces the old 3-pass bf16
#     hi/lo-split emulation at 1/3 the PE cost.
#   - v projection and attn@v run in bf16 (error contribution ~2e-3).
#   - softmax is fp32 (row max on DVE, exp on ACT, normalization deferred).
#
# Layout strategy per core (S=2048, C=1024, P=128):
#   - All transposes (W, x, p) run on the PE (nc.tensor.transpose against an
#     identity): fp32r PE transposes reproduce the fp32r operand pair
#     bit-exactly, so they add no error, and they cost 1.5 cyc/row vs the
#     serialized DMA xbar path which would otherwise bottleneck the kernel.
#   - qT/kT are computed directly transposed ([d, s], d on partitions) so the
#     score matmul contracts over d; v in natural [s, c] layout.
#   - kT (fp32r) and v (bf16) stay resident in SBUF; qT (fp32r) spills to a
#     DRAM scratch and streams back per 128-row query tile.
#   - Attention is software-pipelined: PE order per sq is
#     scores(sq) | pT(sq-1) | attn(sq-1), so softmax (DVE max + ACT exp) of
#     sq overlaps the pT transposes and attn matmuls of sq-1.

from contextlib import ExitStack

import numpy as np

import concourse.bass as bass
import concourse.mybir as mybir
import concourse.tile as tile
from concourse import bacc
from concourse.bass_utils import run_bass_kernel_spmd

F32 = mybir.dt.float32
F32R = mybir.dt.float32r
BF16 = mybir.dt.bfloat16
ADD = mybir.AluOpType.add
AX = mybir.AxisListType.X
EXP = mybir.ActivationFunctionType.Exp

B, S, C = 8, 2048, 1024
P = 128
NT_S = S // P  # 16 s-tiles
NT_C = C // P  # 8 c/d-tiles
CH = 512  # free-dim chunk (one fp32 PSUM bank)
NCH_S = S // CH  # 4
NCH_C = C // CH  # 2


def _emit(tc):
    nc = tc.nc

    x1 = nc.dram_tensor("x1", [S, C], F32R, kind="ExternalInput").ap()
    x2 = nc.dram_tensor("x2", [S, C], F32R, kind="ExternalInput").ap()
    x3 = nc.dram_tensor("x3", [S, C], F32R, kind="ExternalInput").ap()
    Wq = nc.dram_tensor("Wq", [C, C], F32R, kind="ExternalInput").ap()
    Wk = nc.dram_tensor("Wk", [C, C], F32R, kind="ExternalInput").ap()
    Wv = nc.dram_tensor("Wv", [C, C], F32R, kind="ExternalInput").ap()
    bq = nc.dram_tensor("bq", [C], F32, kind="ExternalInput").ap()
    bk = nc.dram_tensor("bk", [C], F32, kind="ExternalInput").ap()
    bv = nc.dram_tensor("bv", [C], F32, kind="ExternalInput").ap()
    eye = nc.dram_tensor("eye", [P, P], F32R, kind="ExternalInput").ap()
    out = nc.dram_tensor("out", [S, C], F32, kind="ExternalOutput").ap()

    es = ExitStack()
    with es:
        const = es.enter_context(tc.tile_pool(name="const", bufs=1))
        dram = es.enter_context(tc.tile_pool(name="dram", bufs=1, space="DRAM"))

        # biases: bq/bk as per-d-tile columns [128, 8]; bv broadcast [128, C]
        bq_sb = const.tile([P, NT_C], F32, tag="bq")
        nc.scalar.dma_start(out=bq_sb, in_=bq.rearrange("(t p) -> p t", p=P))
        bk_sb = const.tile([P, NT_C], F32, tag="bk")
        nc.scalar.dma_start(out=bk_sb, in_=bk.rearrange("(t p) -> p t", p=P))
        bv_sb = const.tile([P, C], F32, tag="bv")
        bv_bcast = bass.AP(tensor=bv.tensor, offset=bv.offset, ap=[[0, P], [1, C]])
        nc.scalar.dma_start(out=bv_sb, in_=bv_bcast)
        eye_r = const.tile([P, P], F32R, tag="eyer")
        nc.scalar.dma_start(out=eye_r, in_=eye)
        eye_h = const.tile([P, P], BF16, tag="eyeh")
        nc.vector.tensor_copy(out=eye_h, in_=eye_r)

        # DRAM scratch for spilled qT
        qT_d = dram.tile([NT_C, P, S], F32R, tag="qTd", name="qTd")

        # resident kT (fp32r) and v (bf16)
        res_k = es.enter_context(tc.tile_pool(name="resk", bufs=1))
        kT = [
            res_k.tile([P, S], F32R, tag=f"kT{i}", name=f"kT{i}")
            for i in range(NT_C)
        ]
        res_v = es.enter_context(tc.tile_pool(name="resv", bufs=1))
        v_res = [
            res_v.tile([P, C], BF16, tag=f"v{i}", name=f"v{i}") for i in range(NT_S)
        ]

        # ---------------- projection phases ----------------------------------
        with (
            tc.tile_pool(name="w", bufs=1) as wpool,
            tc.tile_pool(name="xst", bufs=2) as xst,
            tc.tile_pool(name="xTp", bufs=2) as xTp,
            tc.tile_pool(name="trps", bufs=2, space="PSUM") as trps,
            tc.tile_pool(name="pps", bufs=2, space="PSUM") as pps,
        ):

            def prep_w_row(W, WT, dt):
                """Load W row-tile dt and PE-transpose it into WT[:, :, dt*P:]."""
                wrow = xst.tile([P, C], F32R, tag="xrow", name="wrow")
                nc.sync.dma_start(out=wrow, in_=W[dt * P : (dt + 1) * P, :])
                ps = trps.tile([P, NT_C, P], F32R, tag="tr", name="wtr")
                for ct in range(NT_C):
                    nc.tensor.transpose(
                        ps[:, ct, :], wrow[:, ct * P : (ct + 1) * P], eye_r
                    )
                nc.scalar.copy(out=WT[:, :, dt * P : (dt + 1) * P], in_=ps)

            def stage_x_row(x, xT, s0, j):
                """Load x row-tile at s0+j*P and PE-transpose into xT[:, :, j*P:]."""
                xrow = xst.tile([P, C], F32R, tag="xrow", name="xrow")
                nc.sync.dma_start(out=xrow, in_=x[s0 + j * P : s0 + (j + 1) * P, :])
                ps = trps.tile([P, NT_C, P], F32R, tag="tr", name="xtr")
                for ct in range(NT_C):
                    nc.tensor.transpose(
                        ps[:, ct, :], xrow[:, ct * P : (ct + 1) * P], eye_r
                    )
                nc.scalar.copy(out=xT[:, :, j * P : (j + 1) * P], in_=ps)

            # generic projection phase driver.
            #  kind 'q': dt-groups, sink = qsp -> DRAM spill
            #  kind 'k': dt-groups, sink = resident kT
            #  kind 'v': (j, cch)-groups with x3T stationary, sink = v_res bf16
            def proj_phase(x, W, WT, kind, qsp=None):
                xT_tiles = [None] * (NCH_S + 1)
                xT_tiles[0] = xTp.tile([P, NT_C, CH], F32R, tag="xT", name="xT0")
                for j in range(CH // P):
                    stage_x_row(x, xT_tiles[0], 0, j)
                if kind != "v":
                    # W rows pipeline with the first chunk's dt-groups
                    prep_w_row(W, WT, 0)
                else:
                    for dt in range(NT_C):
                        prep_w_row(W, WT, dt)

                for ich in range(NCH_S):
                    s0 = ich * CH
                    xT = xT_tiles[ich]
                    groups = list(range(NT_C))
                    for g in groups:
                        if kind == "v":
                            j, cch = g // NCH_C, g % NCH_C
                            st = ich * (CH // P) + j
                            ps = pps.tile([P, CH], F32, tag="pps", name="vps")
                            for ct in range(NT_C):
                                nc.tensor.matmul(
                                    ps,
                                    xT[:, ct, j * P : (j + 1) * P],
                                    WT[:, ct, cch * CH : (cch + 1) * CH],
                                    start=(ct == 0),
                                    stop=(ct == NT_C - 1),
                                )
                            nc.vector.tensor_tensor(
                                out=v_res[st][:, cch * CH : (cch + 1) * CH],
                                in0=ps,
                                in1=bv_sb[:, cch * CH : (cch + 1) * CH],
                                op=ADD,
                            )
                        else:
                            dt = g
                            ps = pps.tile([P, CH], F32, tag="pps", name="qkps")
                            for ct in range(NT_C):
                                nc.tensor.matmul(
                                    ps,
                                    WT[:, ct, dt * P : (dt + 1) * P],
                                    xT[:, ct, :],
                                    start=(ct == 0),
                                    stop=(ct == NT_C - 1),
                                )
                            bcol = bq_sb if kind == "q" else bk_sb
                            if kind == "q":
                                t = qsp.tile([P, CH], F32R, tag="qsp", name="qsp")
                                nc.vector.tensor_scalar_add(
                                    out=t, in0=ps, scalar1=bcol[:, dt : dt + 1]
                                )
                                nc.gpsimd.dma_start(
                                    out=qT_d[dt, :, s0 : s0 + CH], in_=t
                                )
                            else:
                                nc.vector.tensor_scalar_add(
                                    out=kT[dt][:, s0 : s0 + CH],
                                    in0=ps,
                                    scalar1=bcol[:, dt : dt + 1],
                                )
                        # interleave hooks (emitted after this MM group):
                        if kind != "v" and ich == 0 and g < NT_C - 1:
                            prep_w_row(W, WT, g + 1)
                        if ich < NCH_S - 1 and NT_C // 2 <= g:
                            # stage next chunk, one row per group
                            j = g - NT_C // 2
                            if j == 0:
                                xT_tiles[ich + 1] = xTp.tile(
                                    [P, NT_C, CH], F32R, tag="xT", name=f"xT{ich + 1}"
                                )
                            stage_x_row(x, xT_tiles[ich + 1], (ich + 1) * CH, j)

            with tc.tile_pool(name="qsp", bufs=2) as qsp:
                WqT = wpool.tile([P, NT_C, C], F32R, tag="WT", name="WqT")
                proj_phase(x1, Wq, WqT, "q", qsp=qsp)
            WkT = wpool.tile([P, NT_C, C], F32R, tag="WT", name="WkT")
            proj_phase(x2, Wk, WkT, "k")
            WvT = wpool.tile([P, NT_C, C], F32R, tag="WT", name="WvT")
            proj_phase(x3, Wv, WvT, "v")

        # ---------------- Attention ------------------------------------------
        with (
            tc.tile_pool(name="qstream", bufs=2) as qstream,
            tc.tile_pool(name="attn", bufs=2) as attn,
            tc.tile_pool(name="stats", bufs=2) as stats,
            tc.tile_pool(name="sps", bufs=1, space="PSUM") as spsum,
            tc.tile_pool(name="ptps", bufs=1, space="PSUM") as ptpsum,
            tc.tile_pool(name="ops", bufs=1, space="PSUM") as opsum,
        ):
            def load_q(sq):
                qt = qstream.tile([P, NT_C, P], F32R, tag="q", name=f"q{sq}")
                nc.sync.dma_start(
                    out=qt,
                    in_=qT_d[:, :, sq * P : (sq + 1) * P].rearrange("t p s -> p t s"),
                )
                return qt

            def scores(sq, qt):
                """Emit score MMs + softmax stats + exp for sq; returns state."""
                ps_s = spsum.tile([P, S], F32, tag="s", name=f"s{sq}")
                mx = stats.tile([P, NCH_S], F32, tag="mx", name="mx")
                for c in range(NCH_S):
                    sl = ps_s[:, c * CH : (c + 1) * CH]
                    for dt in range(NT_C):
                        nc.tensor.matmul(
                            sl,
                            qt[:, dt, :],
                            kT[dt][:, c * CH : (c + 1) * CH],
                            start=(dt == 0),
                            stop=(dt == NT_C - 1),
                        )
                    nc.vector.reduce_max(out=mx[:, c : c + 1], in_=sl, axis=AX)
                negmax = stats.tile([P, 1], F32, tag="negmax", name="negmax")
                nc.vector.reduce_max(out=negmax, in_=mx, axis=AX, negate=True)
                p_sb = attn.tile([P, S], BF16, tag="p", name="p")
                sums = stats.tile([P, NCH_S], F32, tag="sums", name="sums")
                for c in range(NCH_S):
                    nc.scalar.activation(
                        out=p_sb[:, c * CH : (c + 1) * CH],
                        in_=ps_s[:, c * CH : (c + 1) * CH],
                        func=EXP,
                        bias=negmax,
                        scale=1.0,
                        accum_out=sums[:, c : c + 1],
                    )
                rs = stats.tile([P, 1], F32, tag="rs", name="rs")
                nc.vector.reduce_sum(out=rs, in_=sums, axis=AX)
                rinv = stats.tile([P, 1], F32, tag="rinv", name="rinv")
                nc.vector.reciprocal(out=rinv, in_=rs)
                return p_sb, rinv

            def attend(sq, p_sb, rinv):
                """Emit pT transposes + attn MMs + normalize + store for sq."""
                pt_ps = ptpsum.tile([P, NT_S, P], BF16, tag="pt", name="ptps")
                for skt in range(NT_S):
                    nc.tensor.transpose(
                        pt_ps[:, skt, :], p_sb[:, skt * P : (skt + 1) * P], eye_h
                    )
                pT = attn.tile([P, NT_S, P], BF16, tag="pT", name="pT")
                nc.vector.tensor_copy(out=pT, in_=pt_ps)
                ps_o = opsum.tile([P, C], F32, tag="o", name="ops")
                for cch in range(NCH_C):
                    sl = ps_o[:, cch * CH : (cch + 1) * CH]
                    for skt in range(NT_S):
                        nc.tensor.matmul(
                            sl,
                            pT[:, skt, :],
                            v_res[skt][:, cch * CH : (cch + 1) * CH],
                            start=(skt == 0),
                            stop=(skt == NT_S - 1),
                        )
                o_sb = attn.tile([P, C], F32, tag="osb", name="osb")
                nc.vector.tensor_scalar_mul(out=o_sb, in0=ps_o, scalar1=rinv)
                nc.gpsimd.dma_start(out=out[sq * P : (sq + 1) * P, :], in_=o_sb)

            qt = load_q(0)
            prev = None
            for sq in range(NT_S):
                qt_next = load_q(sq + 1) if sq + 1 < NT_S else None
                st = scores(sq, qt)
                if prev is not None:
                    attend(sq - 1, *prev)
                prev = st
                qt = qt_next
            attend(NT_S - 1, *prev)


_BUILT = {}


def _build():
    if "nc" not in _BUILT:
        nc = bacc.Bacc(
            "TRN2",
            target_bir_lowering=False,
            debug=False,
            num_devices=B,
        )
        with tile.TileContext(nc) as tc:
            _emit(tc)
        nc.compile()
        _BUILT["nc"] = nc
    return _BUILT["nc"]


def kernel_with_results(trace=False, **inputs):
    nc = _build()
    eye = np.eye(P, dtype=np.float32)
    in_maps = []
    for i in range(B):
        in_maps.append(
            {
                "x1": np.ascontiguousarray(inputs["x1"][i], dtype=np.float32),
                "x2": np.ascontiguousarray(inputs["x2"][i], dtype=np.float32),
                "x3": np.ascontiguousarray(inputs["x3"][i], dtype=np.float32),
                "Wq": np.ascontiguousarray(inputs["Wq"], dtype=np.float32),
                "Wk": np.ascontiguousarray(inputs["Wk"], dtype=np.float32),
                "Wv": np.ascontiguousarray(inputs["Wv"], dtype=np.float32),
                "bq": np.ascontiguousarray(inputs["bq"], dtype=np.float32),
                "bk": np.ascontiguousarray(inputs["bk"], dtype=np.float32),
                "bv": np.ascontiguousarray(inputs["bv"], dtype=np.float32),
                "eye": eye,
            }
        )
    res = run_bass_kernel_spmd(nc, in_maps, core_ids=list(range(B)), trace=trace)
    outs = np.stack([r["out"] for r in res.results], axis=0).astype(np.float32)
    return outs, res


def kernel(**inputs):
    outs, _ = kernel_with_results(trace=False, **inputs)
    return outs


# revision 9
# speedup vs baseline: 2.5307x; 1.0915x over previous
# Cross-attention kernel for Trainium2 (Bass/Tile), 8-core data-parallel.
#
# Reference computation (per batch element, B=8 -> one batch element per core):
#   q = x1 @ Wq.T + bq ; k = x2 @ Wk.T + bk ; v = x3 @ Wv.T + bv
#   out = softmax(q @ k.T) @ v          (no 1/sqrt(d) scale)
#
# Precision strategy (validated on HW):
#   - q/k/v projections and q@k.T run as single-pass fp32r matmuls. fp32r is
#     the PE's hw bf16 hi/lo pair decomposition of fp32: measured rel err
#     ~1.3e-4 over a K=1024 contraction, and it streams at bf16 speed when
#     the moving free dim is >= 256. This replaces a 3-pass bf16
#     hi/lo-split emulation at 1/3 the PE cost.
#   - attn@v runs in bf16 (v and p stored bf16; error contribution ~2e-3).
#   - softmax is fp32 (row max on DVE, exp on ACT, normalization deferred).
#
# Layout strategy per core (S=2048, C=1024, P=128):
#   - All transposes (W, x, p) run on the PE against an identity: fp32r PE
#     transposes reproduce the fp32r operand pair bit-exactly (measured), so
#     they add no error, and they avoid the serialized DMA xbar path which
#     would otherwise be the bottleneck device.
#   - qT/kT are computed directly transposed ([d, s], d on partitions) so the
#     score matmul contracts over d; v in natural [s, c] layout.
#   - kT (fp32r) and v (bf16) stay resident in SBUF; qT (fp32r) spills to a
#     DRAM scratch and streams back per 128-row query tile.
#   - Projection phases software-pipeline x-row staging, W-row prep, and the
#     next phase's first chunk + W preloads into interleave slots between
#     matmul groups, keeping the PE queue dense across phase boundaries.
#   - Attention pipelines sq tiles: PE order is scores(sq) | pT(sq-1) |
#     attn(sq-1), so softmax (DVE max + ACT exp) of sq overlaps the pT
#     transposes and attn matmuls of sq-1.

from contextlib import ExitStack

import numpy as np

import concourse.bass as bass
import concourse.mybir as mybir
import concourse.tile as tile
from concourse import bacc
from concourse.bass_utils import run_bass_kernel_spmd

F32 = mybir.dt.float32
F32R = mybir.dt.float32r
BF16 = mybir.dt.bfloat16
ADD = mybir.AluOpType.add
AX = mybir.AxisListType.X
EXP = mybir.ActivationFunctionType.Exp

B, S, C = 8, 2048, 1024
P = 128
NT_S = S // P  # 16 s-tiles
NT_C = C // P  # 8 c/d-tiles
CH = 512  # free-dim chunk (one fp32 PSUM bank)
NCH_S = S // CH  # 4
NCH_C = C // CH  # 2
N_WVPRE = 6  # Wv rows preloaded into the dedicated pool (6..7 go via xst)


def _emit(tc):
    nc = tc.nc

    x1 = nc.dram_tensor("x1", [S, C], F32R, kind="ExternalInput").ap()
    x2 = nc.dram_tensor("x2", [S, C], F32R, kind="ExternalInput").ap()
    x3 = nc.dram_tensor("x3", [S, C], F32R, kind="ExternalInput").ap()
    Wq = nc.dram_tensor("Wq", [C, C], F32R, kind="ExternalInput").ap()
    Wk = nc.dram_tensor("Wk", [C, C], F32R, kind="ExternalInput").ap()
    Wv = nc.dram_tensor("Wv", [C, C], F32R, kind="ExternalInput").ap()
    bq = nc.dram_tensor("bq", [C], F32, kind="ExternalInput").ap()
    bk = nc.dram_tensor("bk", [C], F32, kind="ExternalInput").ap()
    bvh = nc.dram_tensor("bvh", [C], BF16, kind="ExternalInput").ap()
    eye = nc.dram_tensor("eye", [P, P], F32R, kind="ExternalInput").ap()
    out = nc.dram_tensor("out", [S, C], F32, kind="ExternalOutput").ap()

    es = ExitStack()
    with es:
        const = es.enter_context(tc.tile_pool(name="const", bufs=1))
        dram = es.enter_context(tc.tile_pool(name="dram", bufs=1, space="DRAM"))

        eye_r = const.tile([P, P], F32R, tag="eyer")
        nc.scalar.dma_start(out=eye_r, in_=eye)
        eye_h = const.tile([P, P], BF16, tag="eyeh")
        nc.vector.tensor_copy(out=eye_h, in_=eye_r)
        # bias tiles declared now, loaded after the first x/W rows are queued
        bq_sb = const.tile([P, NT_C], F32, tag="bq")
        bk_sb = const.tile([P, NT_C], F32, tag="bk")
        bv_sb = const.tile([P, C], BF16, tag="bv")

        def load_biases():
            nc.scalar.dma_start(out=bq_sb, in_=bq.rearrange("(t p) -> p t", p=P))
            nc.scalar.dma_start(out=bk_sb, in_=bk.rearrange("(t p) -> p t", p=P))
            bv_bc = bass.AP(tensor=bvh.tensor, offset=bvh.offset, ap=[[0, P], [1, C]])
            nc.scalar.dma_start(out=bv_sb, in_=bv_bc)

        # DRAM scratch for spilled qT
        qT_d = dram.tile([NT_C, P, S], F32R, tag="qTd", name="qTd")

        # resident kT (fp32r) and v (bf16)
        res_k = es.enter_context(tc.tile_pool(name="resk", bufs=1))
        kT = [
            res_k.tile([P, S], F32R, tag=f"kT{i}", name=f"kT{i}")
            for i in range(NT_C)
        ]
        res_v = es.enter_context(tc.tile_pool(name="resv", bufs=1))
        v_res = [
            res_v.tile([P, C], BF16, tag=f"v{i}", name=f"v{i}") for i in range(NT_S)
        ]
        qstream = es.enter_context(tc.tile_pool(name="qstream", bufs=2))

        def load_q(sq):
            qt = qstream.tile([P, NT_C, P], F32R, tag="q", name=f"q{sq}")
            nc.sync.dma_start(
                out=qt,
                in_=qT_d[:, :, sq * P : (sq + 1) * P].rearrange("t p s -> p t s"),
            )
            return qt

        # ---------------- projection phases ----------------------------------
        with (
            tc.tile_pool(name="w", bufs=1) as wpool,
            tc.tile_pool(name="xst", bufs=3) as xst,
            tc.tile_pool(name="xTp", bufs=2) as xTp,
            tc.tile_pool(name="trps", bufs=2, space="PSUM") as trps,
            tc.tile_pool(name="pps", bufs=2, space="PSUM") as pps,
        ):
            copy_flip = [0]

            def psum_copy(out_, in_):
                """Alternate psum->sbuf copies between ACT and DVE (the only
                engines that can read PSUM besides the PE)."""
                if copy_flip[0] % 2 == 0:
                    nc.scalar.copy(out=out_, in_=in_)
                else:
                    nc.vector.tensor_copy(out=out_, in_=in_)
                copy_flip[0] += 1

            def transpose_w_row(wrow, WT, dt):
                ps = trps.tile([P, NT_C, P], F32R, tag="tr", name="wtr")
                for ct in range(NT_C):
                    nc.tensor.transpose(
                        ps[:, ct, :], wrow[:, ct * P : (ct + 1) * P], eye_r
                    )
                psum_copy(WT[:, :, dt * P : (dt + 1) * P], ps)

            def prep_w_row(W, WT, dt):
                wrow = xst.tile([P, C], F32R, tag="xrow", name="wrow")
                nc.sync.dma_start(out=wrow, in_=W[dt * P : (dt + 1) * P, :])
                transpose_w_row(wrow, WT, dt)

            def stage_x_row(x, xT, s0, j):
                xrow = xst.tile([P, C], F32R, tag="xrow", name="xrow")
                nc.sync.dma_start(out=xrow, in_=x[s0 + j * P : s0 + (j + 1) * P, :])
                ps = trps.tile([P, NT_C, P], F32R, tag="tr", name="xtr")
                for ct in range(NT_C):
                    nc.tensor.transpose(
                        ps[:, ct, :], xrow[:, ct * P : (ct + 1) * P], eye_r
                    )
                psum_copy(xT[:, :, j * P : (j + 1) * P], ps)

            def new_xT(name):
                return xTp.tile([P, NT_C, CH], F32R, tag="xT", name=name)

            def proj_phase(
                x,
                W,
                WT,
                kind,
                qsp=None,
                xT0=None,
                w_pre=None,
                dma_tasks=(),
                stage_tasks=(),
                after_first_row=None,
                after_w0=None,
            ):
                """One projection phase.

                kind 'q': dt-groups, sink -> DRAM spill (via qsp tiles)
                kind 'k': dt-groups, sink -> resident kT
                kind 'v': (cch, j)-groups, sink -> v_res bf16
                xT0: prestaged chunk-0 tile (else staged here)
                w_pre: dict dt -> preloaded W row tile
                dma_tasks: pure-DMA tasks run at ich2 groups 0,2,4,6
                stage_tasks: tasks run at ich3 groups 0..7
                """
                w_pre = w_pre or {}
                if xT0 is None:
                    xT0 = new_xT("xT0")
                    for j in range(CH // P):
                        stage_x_row(x, xT0, 0, j)
                        if j == 0 and after_first_row is not None:
                            after_first_row()
                            after_first_row = None
                xT_cur = xT0
                if kind == "v":
                    for dt in range(4):
                        transpose_w_row(w_pre[dt], WT, dt)
                elif 0 in w_pre:
                    transpose_w_row(w_pre[0], WT, 0)
                else:
                    prep_w_row(W, WT, 0)
                if after_w0 is not None:
                    after_w0()

                dma_tasks = list(dma_tasks)
                stage_tasks = list(stage_tasks)
                xT_next = None
                for ich in range(NCH_S):
                    s0 = ich * CH
                    xT = xT_cur
                    for g in range(NT_C):
                        if kind == "v":
                            cch, j = g // 4, g % 4
                            st = ich * (CH // P) + j
                            ps = pps.tile([P, CH], F32, tag="pps", name="vps")
                            for ct in range(NT_C):
                                nc.tensor.matmul(
                                    ps,
                                    xT[:, ct, j * P : (j + 1) * P],
                                    WT[:, ct, cch * CH : (cch + 1) * CH],
                                    start=(ct == 0),
                                    stop=(ct == NT_C - 1),
                                )
                            nc.vector.tensor_tensor(
                                out=v_res[st][:, cch * CH : (cch + 1) * CH],
                                in0=ps,
                                in1=bv_sb[:, cch * CH : (cch + 1) * CH],
                                op=ADD,
                            )
                        else:
                            dt = g
                            ps = pps.tile([P, CH], F32, tag="pps", name="qkps")
                            for ct in range(NT_C):
                                nc.tensor.matmul(
                                    ps,
                                    WT[:, ct, dt * P : (dt + 1) * P],
                                    xT[:, ct, :],
                                    start=(ct == 0),
                                    stop=(ct == NT_C - 1),
                                )
                            bcol = bq_sb if kind == "q" else bk_sb
                            if kind == "q":
                                t = qsp.tile([P, CH], F32R, tag="qsp", name="qsp")
                                nc.vector.tensor_scalar_add(
                                    out=t, in0=ps, scalar1=bcol[:, dt : dt + 1]
                                )
                                nc.gpsimd.dma_start(
                                    out=qT_d[dt, :, s0 : s0 + CH], in_=t
                                )
                            else:
                                nc.vector.tensor_scalar_add(
                                    out=kT[dt][:, s0 : s0 + CH],
                                    in0=ps,
                                    scalar1=bcol[:, dt : dt + 1],
                                )
                        # ---- interleave hooks ----
                        if ich == 0:
                            if kind == "v":
                                if g < 4:
                                    transpose_w_row(w_pre[4 + g], WT, 4 + g)
                            elif g < NT_C - 1:
                                prep_w_row(W, WT, g + 1)
                        if ich < NCH_S - 1 and g % 2 == 1:
                            j = (g - 1) // 2
                            if j == 0:
                                xT_next = new_xT(f"xT{ich + 1}")
                            stage_x_row(x, xT_next, (ich + 1) * CH, j)
                            if j == 3:
                                xT_cur = xT_next
                        if ich == 2 and g % 2 == 0 and dma_tasks:
                            dma_tasks.pop(0)()
                        if ich == 3 and stage_tasks:
                            stage_tasks.pop(0)()

            # ---- task construction ----
            holders = {}

            def stage_task(x, hkey, j):
                def t():
                    if hkey not in holders:
                        holders[hkey] = new_xT(hkey)
                    stage_x_row(x, holders[hkey], 0, j)

                return t

            wpre_cm = tc.tile_pool(name="wpre", bufs=1)
            wpre_pool = wpre_cm.__enter__()
            wvpre_cm = tc.tile_pool(name="wvpre", bufs=1)

            def load_wk0():
                holders["wk0"] = wpre_pool.tile([P, C], F32R, tag="wk0", name="wk0")
                nc.sync.dma_start(out=holders["wk0"], in_=Wk[0:P, :])

            def load_wv(i):
                def t():
                    if "wvpre" not in holders:
                        pool = wvpre_cm.__enter__()
                        holders["wvpre"] = pool.tile(
                            [P, N_WVPRE, C], F32R, tag="wvpre", name="wvpre"
                        )
                    if i < N_WVPRE:
                        nc.sync.dma_start(
                            out=holders["wvpre"][:, i, :], in_=Wv[i * P : (i + 1) * P, :]
                        )
                    else:
                        w = xst.tile([P, C], F32R, tag="xrow", name=f"wv{i}")
                        nc.sync.dma_start(out=w, in_=Wv[i * P : (i + 1) * P, :])
                        holders[f"wv{i}"] = w

                return t

            def qpre(sq):
                def t():
                    holders[f"qt{sq}"] = load_q(sq)

                return t

            # ---- run the three phases ----
            with tc.tile_pool(name="qsp", bufs=2) as qsp:
                WqT = wpool.tile([P, NT_C, C], F32R, tag="WT", name="WqT")
                proj_phase(
                    x1,
                    Wq,
                    WqT,
                    "q",
                    qsp=qsp,
                    after_first_row=load_biases,
                    stage_tasks=[stage_task(x2, "x2c0", j) for j in range(4)]
                    + [load_wk0],
                )
            WkT = wpool.tile([P, NT_C, C], F32R, tag="WT", name="WkT")
            proj_phase(
                x2,
                Wk,
                WkT,
                "k",
                xT0=holders["x2c0"],
                w_pre={0: holders["wk0"]},
                after_w0=lambda: wpre_cm.__exit__(None, None, None),
                dma_tasks=[load_wv(i) for i in range(4)],
                stage_tasks=[stage_task(x3, "x3c0", j) for j in range(4)]
                + [load_wv(i) for i in range(4, NT_C)],
            )
            wv_rows = {
                i: (holders["wvpre"][:, i, :] if i < N_WVPRE else holders[f"wv{i}"])
                for i in range(NT_C)
            }
            WvT = wpool.tile([P, NT_C, C], F32R, tag="WT", name="WvT")
            proj_phase(
                x3,
                Wv,
                WvT,
                "v",
                xT0=holders["x3c0"],
                w_pre=wv_rows,
                stage_tasks=[qpre(0), qpre(1)],
            )
            wvpre_cm.__exit__(None, None, None)

        # ---------------- Attention ------------------------------------------
        with (
            tc.tile_pool(name="attn", bufs=2) as attn,
            tc.tile_pool(name="stats", bufs=2) as stats,
            tc.tile_pool(name="sps", bufs=1, space="PSUM") as spsum,
            tc.tile_pool(name="ptps", bufs=1, space="PSUM") as ptpsum,
            tc.tile_pool(name="ops", bufs=1, space="PSUM") as opsum,
        ):
            def scores(sq, qt):
                ps_s = spsum.tile([P, S], F32, tag="s", name=f"s{sq}")
                mx = stats.tile([P, NCH_S], F32, tag="mx", name="mx")
                for c in range(NCH_S):
                    sl = ps_s[:, c * CH : (c + 1) * CH]
                    for dt in range(NT_C):
                        nc.tensor.matmul(
                            sl,
                            qt[:, dt, :],
                            kT[dt][:, c * CH : (c + 1) * CH],
                            start=(dt == 0),
                            stop=(dt == NT_C - 1),
                        )
                    nc.vector.reduce_max(out=mx[:, c : c + 1], in_=sl, axis=AX)
                negmax = stats.tile([P, 1], F32, tag="negmax", name="negmax")
                nc.vector.reduce_max(out=negmax, in_=mx, axis=AX, negate=True)
                p_sb = attn.tile([P, S], BF16, tag="p", name="p")
                sums = stats.tile([P, NCH_S], F32, tag="sums", name="sums")
                for c in range(NCH_S):
                    nc.scalar.activation(
                        out=p_sb[:, c * CH : (c + 1) * CH],
                        in_=ps_s[:, c * CH : (c + 1) * CH],
                        func=EXP,
                        bias=negmax,
                        scale=1.0,
                        accum_out=sums[:, c : c + 1],
                    )
                rs = stats.tile([P, 1], F32, tag="rs", name="rs")
                nc.vector.reduce_sum(out=rs, in_=sums, axis=AX)
                rinv = stats.tile([P, 1], F32, tag="rinv", name="rinv")
                nc.vector.reciprocal(out=rinv, in_=rs)
                return p_sb, rinv

            def attend(sq, p_sb, rinv):
                pt_ps = ptpsum.tile([P, NT_S, P], BF16, tag="pt", name="ptps")
                for skt in range(NT_S):
                    nc.tensor.transpose(
                        pt_ps[:, skt, :], p_sb[:, skt * P : (skt + 1) * P], eye_h
                    )
                pT = attn.tile([P, NT_S, P], BF16, tag="pT", name="pT")
                nc.vector.tensor_copy(out=pT, in_=pt_ps)
                ps_o = opsum.tile([P, C], F32, tag="o", name="ops")
                for cch in range(NCH_C):
                    sl = ps_o[:, cch * CH : (cch + 1) * CH]
                    for skt in range(NT_S):
                        nc.tensor.matmul(
                            sl,
                            pT[:, skt, :],
                            v_res[skt][:, cch * CH : (cch + 1) * CH],
                            start=(skt == 0),
                            stop=(skt == NT_S - 1),
                        )
                o_sb = attn.tile([P, C], F32, tag="osb", name="osb")
                for cch in range(NCH_C):
                    nc.vector.tensor_scalar_mul(
                        out=o_sb[:, cch * CH : (cch + 1) * CH],
                        in0=ps_o[:, cch * CH : (cch + 1) * CH],
                        scalar1=rinv,
                    )
                    nc.gpsimd.dma_start(
                        out=out[sq * P : (sq + 1) * P, cch * CH : (cch + 1) * CH],
                        in_=o_sb[:, cch * CH : (cch + 1) * CH],
                    )

            qts = {0: holders["qt0"], 1: holders["qt1"]}
            prev = None
            for sq in range(NT_S):
                if sq + 2 < NT_S:
                    qts[sq + 2] = load_q(sq + 2)
                st = scores(sq, qts.pop(sq))
                if prev is not None:
                    attend(sq - 1, *prev)
                prev = st
            attend(NT_S - 1, *prev)


_BUILT = {}


def _build():
    if "nc" not in _BUILT:
        nc = bacc.Bacc(
            "TRN2",
            target_bir_lowering=False,
            debug=False,
            num_devices=B,
        )
        with tile.TileContext(nc) as tc:
            _emit(tc)
        nc.compile()
        _BUILT["nc"] = nc
    return _BUILT["nc"]


def kernel_with_results(trace=False, **inputs):
    import ml_dtypes

    nc = _build()
    eye = np.eye(P, dtype=np.float32)
    bvh = np.asarray(inputs["bv"], dtype=np.float32).astype(ml_dtypes.bfloat16)
    in_maps = []
    for i in range(B):
        in_maps.append(
            {
                "x1": np.ascontiguousarray(inputs["x1"][i], dtype=np.float32),
                "x2": np.ascontiguousarray(inputs["x2"][i], dtype=np.float32),
                "x3": np.ascontiguousarray(inputs["x3"][i], dtype=np.float32),
                "Wq": np.ascontiguousarray(inputs["Wq"], dtype=np.float32),
                "Wk": np.ascontiguousarray(inputs["Wk"], dtype=np.float32),
                "Wv": np.ascontiguousarray(inputs["Wv"], dtype=np.float32),
                "bq": np.ascontiguousarray(inputs["bq"], dtype=np.float32),
                "bk": np.ascontiguousarray(inputs["bk"], dtype=np.float32),
                "bvh": bvh,
                "eye": eye,
            }
        )
    res = run_bass_kernel_spmd(nc, in_maps, core_ids=list(range(B)), trace=trace)
    outs = np.stack([r["out"] for r in res.results], axis=0).astype(np.float32)
    return outs, res


def kernel(**inputs):
    outs, _ = kernel_with_results(trace=False, **inputs)
    return outs


# revision 15
# speedup vs baseline: 2.5829x; 1.0206x over previous
# Cross-attention kernel for Trainium2 (Bass/Tile), 8-core data-parallel.
#
# Reference computation (per batch element, B=8 -> one batch element per core):
#   q = x1 @ Wq.T + bq ; k = x2 @ Wk.T + bk ; v = x3 @ Wv.T + bv
#   out = softmax(q @ k.T) @ v          (no 1/sqrt(d) scale)
#
# Precision strategy (validated on HW):
#   - q/k/v projections and q@k.T run as single-pass fp32r matmuls. fp32r is
#     the PE's hw bf16 hi/lo pair decomposition of fp32: measured rel err
#     ~1.3e-4 over a K=1024 contraction, and it streams at bf16 speed when
#     the moving free dim is >= 256. This replaces a 3-pass bf16
#     hi/lo-split emulation at 1/3 the PE cost.
#   - attn@v runs in bf16 (v and p stored bf16; error contribution ~2e-3).
#   - softmax is fp32 (row max on DVE, exp on ACT, normalization deferred).
#
# Layout strategy per core (S=2048, C=1024, P=128):
#   - All transposes (W, x, p) run on the PE against an identity: fp32r PE
#     transposes reproduce the fp32r operand pair bit-exactly (measured), so
#     they add no error, and they avoid the serialized DMA xbar path which
#     would otherwise be the bottleneck device.
#   - qT/kT are computed directly transposed ([d, s], d on partitions) so the
#     score matmul contracts over d; v in natural [s, c] layout.
#   - kT (fp32r) and v (bf16) stay resident in SBUF; qT (fp32r) spills to a
#     DRAM scratch and streams back per 128-row query tile.
#   - Projection phases software-pipeline x-row staging, W-row prep, and the
#     next phase's first chunk + W preloads into interleave slots between
#     matmul groups, keeping the PE queue dense across phase boundaries.
#   - Attention pipelines sq tiles: PE order is scores(sq) | pT(sq-1) |
#     attn(sq-1), so softmax (DVE max + ACT exp) of sq overlaps the pT
#     transposes and attn matmuls of sq-1.

from contextlib import ExitStack

import numpy as np

import concourse.bass as bass
import concourse.mybir as mybir
import concourse.tile as tile
from concourse import bacc
from concourse.bass_utils import run_bass_kernel_spmd

F32 = mybir.dt.float32
F32R = mybir.dt.float32r
BF16 = mybir.dt.bfloat16
ADD = mybir.AluOpType.add
MUL = mybir.AluOpType.mult
AX = mybir.AxisListType.X
EXP = mybir.ActivationFunctionType.Exp
COPY = mybir.ActivationFunctionType.Copy

B, S, C = 8, 2048, 1024
P = 128
NT_S = S // P  # 16 s-tiles
NT_C = C // P  # 8 c/d-tiles
CH = 512  # free-dim chunk (one fp32 PSUM bank)
NCH_S = S // CH  # 4
NCH_C = C // CH  # 2
N_WVPRE = 6  # Wv rows preloaded into the dedicated pool (6..7 go via xst)


def _emit(tc):
    nc = tc.nc

    x1 = nc.dram_tensor("x1", [S, C], F32R, kind="ExternalInput").ap()
    x2 = nc.dram_tensor("x2", [S, C], F32R, kind="ExternalInput").ap()
    x3 = nc.dram_tensor("x3", [S, C], F32R, kind="ExternalInput").ap()
    Wq = nc.dram_tensor("Wq", [C, C], F32R, kind="ExternalInput").ap()
    Wk = nc.dram_tensor("Wk", [C, C], F32R, kind="ExternalInput").ap()
    Wv = nc.dram_tensor("Wv", [C, C], F32R, kind="ExternalInput").ap()
    bq = nc.dram_tensor("bq", [C], F32, kind="ExternalInput").ap()
    bk = nc.dram_tensor("bk", [C], F32, kind="ExternalInput").ap()
    bvh = nc.dram_tensor("bvh", [C], BF16, kind="ExternalInput").ap()
    eye = nc.dram_tensor("eye", [P, P], F32R, kind="ExternalInput").ap()
    out = nc.dram_tensor("out", [S, C], F32, kind="ExternalOutput").ap()

    es = ExitStack()
    with es:
        const = es.enter_context(tc.tile_pool(name="const", bufs=1))
        dram = es.enter_context(tc.tile_pool(name="dram", bufs=1, space="DRAM"))

        eye_r = const.tile([P, P], F32R, tag="eyer")
        nc.scalar.dma_start(out=eye_r, in_=eye)
        eye_h = const.tile([P, P], BF16, tag="eyeh")
        nc.vector.tensor_copy(out=eye_h, in_=eye_r)
        # bias tiles declared now, loaded after the first x/W rows are queued
        bq_sb = const.tile([P, NT_C], F32, tag="bq")
        bk_sb = const.tile([P, NT_C], F32, tag="bk")
        bv_sb = const.tile([P, C], BF16, tag="bv")

        def load_biases():
            nc.gpsimd.dma_start(out=bq_sb, in_=bq.rearrange("(t p) -> p t", p=P))
            nc.gpsimd.dma_start(out=bk_sb, in_=bk.rearrange("(t p) -> p t", p=P))
            bv_bc = bass.AP(tensor=bvh.tensor, offset=bvh.offset, ap=[[0, P], [1, C]])
            nc.gpsimd.dma_start(out=bv_sb, in_=bv_bc)

        # DRAM scratch for spilled qT
        qT_d = dram.tile([NT_C, P, S], F32R, tag="qTd", name="qTd")

        # resident kT (fp32r) and v (bf16)
        res_k = es.enter_context(tc.tile_pool(name="resk", bufs=1))
        kT = [
            res_k.tile([P, S], F32R, tag=f"kT{i}", name=f"kT{i}")
            for i in range(NT_C)
        ]
        res_v = es.enter_context(tc.tile_pool(name="resv", bufs=1))
        v_res = [
            res_v.tile([P, C], BF16, tag=f"v{i}", name=f"v{i}") for i in range(NT_S)
        ]
        qstream = es.enter_context(tc.tile_pool(name="qstream", bufs=2))

        def load_q(sq):
            qt = qstream.tile([P, NT_C, P], F32R, tag="q", name=f"q{sq}")
            nc.sync.dma_start(
                out=qt,
                in_=qT_d[:, :, sq * P : (sq + 1) * P].rearrange("t p s -> p t s"),
            )
            return qt

        # ---------------- projection phases ----------------------------------
        with (
            tc.tile_pool(name="w", bufs=1) as wpool,
            tc.tile_pool(name="xst", bufs=3) as xst,
            tc.tile_pool(name="xTp", bufs=2) as xTp,
            tc.tile_pool(name="trps", bufs=2, space="PSUM") as trps,
            tc.tile_pool(name="pps", bufs=2, space="PSUM") as pps,
        ):
            H = NT_C // 2
            load_flip = [0]

            def row_load(dst, src):
                """Alternate row loads between the HWDGE (sync) and SWDGE
                (gpsimd) descriptor generators."""
                eng = nc.sync if load_flip[0] % 2 == 0 else nc.gpsimd
                load_flip[0] += 1
                eng.dma_start(out=dst, in_=src)

            def transpose_w_row(wrow, WT, dt):
                ps = trps.tile([P, NT_C, P], F32R, tag="tr", name="wtr")
                for ct in range(NT_C):
                    nc.tensor.transpose(
                        ps[:, ct, :], wrow[:, ct * P : (ct + 1) * P], eye_r
                    )
                nc.scalar.copy(
                    out=WT[:, 0:H, dt * P : (dt + 1) * P], in_=ps[:, 0:H, :]
                )
                nc.vector.tensor_copy(
                    out=WT[:, H:NT_C, dt * P : (dt + 1) * P], in_=ps[:, H:NT_C, :]
                )

            def prep_w_row(W, WT, dt):
                wrow = xst.tile([P, C], F32R, tag="xrow", name="wrow")
                row_load(wrow, W[dt * P : (dt + 1) * P, :])
                transpose_w_row(wrow, WT, dt)

            def stage_x_row(x, xT, s0, j):
                xrow = xst.tile([P, C], F32R, tag="xrow", name="xrow")
                row_load(xrow, x[s0 + j * P : s0 + (j + 1) * P, :])
                ps = trps.tile([P, NT_C, P], F32R, tag="tr", name="xtr")
                for ct in range(NT_C):
                    nc.tensor.transpose(
                        ps[:, ct, :], xrow[:, ct * P : (ct + 1) * P], eye_r
                    )
                nc.scalar.copy(
                    out=xT[:, 0:H, j * P : (j + 1) * P], in_=ps[:, 0:H, :]
                )
                nc.vector.tensor_copy(
                    out=xT[:, H:NT_C, j * P : (j + 1) * P], in_=ps[:, H:NT_C, :]
                )

            def new_xT(name):
                return xTp.tile([P, NT_C, CH], F32R, tag="xT", name=name)

            def proj_phase(
                x,
                W,
                WT,
                kind,
                qsp=None,
                xT0=None,
                w_pre=None,
                dma_tasks=(),
                stage_tasks=(),
                flush_tasks=(),
                defer_spills=None,
                after_first_row=None,
                after_w0=None,
                w_done=0,
            ):
                """One projection phase.

                kind 'q': dt-groups, sink -> DRAM spill (via qsp tiles)
                kind 'k': dt-groups, sink -> resident kT
                kind 'v': (cch, j)-groups, sink -> v_res bf16
                xT0: prestaged chunk-0 tile (else staged here)
                w_pre: dict dt -> preloaded W row tile
                dma_tasks: pure-DMA tasks run at ich2 groups 0,2,4,6
                stage_tasks: tasks run at ich3 groups 0..7
                """
                w_pre = w_pre or {}
                if xT0 is None:
                    xT0 = new_xT("xT0")
                    for j in range(CH // P):
                        stage_x_row(x, xT0, 0, j)
                        if j == 0 and after_first_row is not None:
                            after_first_row()
                            after_first_row = None
                xT_cur = xT0
                if w_done == 0:
                    prep_w_row(W, WT, 0)
                if after_w0 is not None:
                    after_w0()

                dma_tasks = list(dma_tasks)
                flush_tasks = list(flush_tasks)
                xT_next = None
                for ich in range(NCH_S):
                    s0 = ich * CH
                    xT = xT_cur
                    for g in range(NT_C):
                        if kind == "v":
                            cch, j = g // 4, g % 4
                            st = ich * (CH // P) + j
                            ps = pps.tile([P, CH], F32, tag="pps", name="vps")
                            for ct in range(NT_C):
                                nc.tensor.matmul(
                                    ps,
                                    xT[:, ct, j * P : (j + 1) * P],
                                    WT[:, ct, cch * CH : (cch + 1) * CH],
                                    start=(ct == 0),
                                    stop=(ct == NT_C - 1),
                                )
                            hch = CH // 2
                            nc.scalar.copy(
                                out=v_res[st][:, cch * CH : cch * CH + hch],
                                in_=ps[:, 0:hch],
                            )
                            nc.vector.tensor_copy(
                                out=v_res[st][:, cch * CH + hch : (cch + 1) * CH],
                                in_=ps[:, hch:CH],
                            )
                        else:
                            dt = g
                            ps = pps.tile([P, CH], F32, tag="pps", name="qkps")
                            for ct in range(NT_C):
                                nc.tensor.matmul(
                                    ps,
                                    WT[:, ct, dt * P : (dt + 1) * P],
                                    xT[:, ct, :],
                                    start=(ct == 0),
                                    stop=(ct == NT_C - 1),
                                )
                            bcol = bq_sb if kind == "q" else bk_sb
                            if kind == "q":
                                t = qsp.tile([P, CH], F32R, tag="qsp", name="qsp")
                                nc.vector.tensor_scalar_add(
                                    out=t, in0=ps, scalar1=bcol[:, dt : dt + 1]
                                )

                                def spill(t=t, dt=dt, s0=s0):
                                    nc.gpsimd.dma_start(
                                        out=qT_d[dt, :, s0 : s0 + CH], in_=t
                                    )

                                if defer_spills is not None and ich == NCH_S - 1:
                                    defer_spills.append(spill)
                                else:
                                    spill()
                            else:
                                nc.vector.tensor_scalar_add(
                                    out=kT[dt][:, s0 : s0 + CH],
                                    in0=ps,
                                    scalar1=bcol[:, dt : dt + 1],
                                )
                        # ---- interleave hooks ----
                        if ich == 0:
                            if kind == "v":
                                if w_done + g < NT_C:
                                    transpose_w_row(w_pre[w_done + g], WT, w_done + g)
                            elif g < NT_C - 1:
                                prep_w_row(W, WT, g + 1)
                        if ich < NCH_S - 1 and g % 2 == 1:
                            j = (g - 1) // 2
                            if j == 0:
                                xT_next = new_xT(f"xT{ich + 1}")
                            stage_x_row(x, xT_next, (ich + 1) * CH, j)
                            if j == 3:
                                xT_cur = xT_next
                        if ich in (0, 1) and g % 2 == 0 and flush_tasks:
                            flush_tasks.pop(0)()
                        if ich == 2 and g % 2 == 0 and dma_tasks:
                            dma_tasks.pop(0)()
                        if ich == 3 and g < len(stage_tasks):
                            for t in stage_tasks[g]:
                                t()

            # ---- task construction ----
            holders = {}

            def stage_task(x, hkey, j):
                def t():
                    if hkey not in holders:
                        holders[hkey] = new_xT(hkey)
                    stage_x_row(x, holders[hkey], 0, j)

                return t

            qsp_cm = tc.tile_pool(name="qsp", bufs=8)
            qsp = qsp_cm.__enter__()
            wpre_cm = tc.tile_pool(name="wpre", bufs=1)
            wpre_pool = wpre_cm.__enter__()

            def load_wk0():
                holders["wk0"] = wpre_pool.tile([P, C], F32R, tag="wk0", name="wk0")
                nc.sync.dma_start(out=holders["wk0"], in_=Wk[0:P, :])

            def load_wv(i):
                def t():
                    w = xst.tile([P, C], F32R, tag="xrow", name=f"wv{i}")
                    row_load(w, Wv[i * P : (i + 1) * P, :])
                    holders[f"wv{i}"] = w

                return t

            def qpre(sq):
                def t():
                    holders[f"qt{sq}"] = load_q(sq)

                return t

            # ---- run the three phases ----
            WqT = wpool.tile([P, NT_C, C], F32R, tag="WT", name="WqT")
            WkT = wpool.tile([P, NT_C, C], F32R, tag="WT", name="WkT")
            WvT = wpool.tile([P, NT_C, C], F32R, tag="WT", name="WvT")

            def tr_wk0():
                transpose_w_row(holders["wk0"], WkT, 0)

            def tr_wv(r):
                def t():
                    transpose_w_row(holders[f"wv{r}"], WvT, r)

                return t

            deferred = []
            proj_phase(
                x1,
                Wq,
                WqT,
                "q",
                qsp=qsp,
                defer_spills=deferred,
                after_first_row=load_biases,
                stage_tasks=[
                    [stage_task(x2, "x2c0", 0)],
                    [stage_task(x2, "x2c0", 1)],
                    [stage_task(x2, "x2c0", 2)],
                    [stage_task(x2, "x2c0", 3)],
                    [load_wk0],
                    [tr_wk0],
                ],
            )
            proj_phase(
                x2,
                Wk,
                WkT,
                "k",
                xT0=holders["x2c0"],
                w_done=1,
                after_w0=lambda: wpre_cm.__exit__(None, None, None),
                flush_tasks=deferred,
                stage_tasks=[
                    [stage_task(x3, "x3c0", 0)],
                    [stage_task(x3, "x3c0", 1), load_wv(0)],
                    [stage_task(x3, "x3c0", 2), tr_wv(0), load_wv(1)],
                    [stage_task(x3, "x3c0", 3), tr_wv(1), load_wv(2)],
                    [tr_wv(2), load_wv(3)],
                    [tr_wv(3), load_wv(4)],
                    [tr_wv(4), load_wv(5)],
                    [tr_wv(5)],
                ],
            )
            qsp_cm.__exit__(None, None, None)
            load_wv(6)()
            load_wv(7)()
            wv_rows = {6: holders["wv6"], 7: holders["wv7"]}
            proj_phase(
                x3,
                Wv,
                WvT,
                "v",
                xT0=holders["x3c0"],
                w_pre=wv_rows,
                w_done=6,
                stage_tasks=[[qpre(0)], [qpre(1)]],
            )

        # ---------------- Attention ------------------------------------------
        with (
            tc.tile_pool(name="attn", bufs=2) as attn,
            tc.tile_pool(name="stats", bufs=2) as stats,
            tc.tile_pool(name="sps", bufs=1, space="PSUM") as spsum,
            tc.tile_pool(name="ptps", bufs=1, space="PSUM") as ptpsum,
            tc.tile_pool(name="ops", bufs=1, space="PSUM") as opsum,
        ):
            def scores(sq, qt):
                ps_s = spsum.tile([P, S], F32, tag="s", name=f"s{sq}")
                mx = stats.tile([P, NCH_S], F32, tag="mx", name="mx")
                for c in range(NCH_S):
                    sl = ps_s[:, c * CH : (c + 1) * CH]
                    for dt in range(NT_C):
                        nc.tensor.matmul(
                            sl,
                            qt[:, dt, :],
                            kT[dt][:, c * CH : (c + 1) * CH],
                            start=(dt == 0),
                            stop=(dt == NT_C - 1),
                        )
                    nc.vector.reduce_max(out=mx[:, c : c + 1], in_=sl, axis=AX)
                negmax = stats.tile([P, 1], F32, tag="negmax", name="negmax")
                nc.vector.reduce_max(out=negmax, in_=mx, axis=AX, negate=True)
                p_sb = attn.tile([P, S], BF16, tag="p", name="p")
                sums = stats.tile([P, NCH_S], F32, tag="sums", name="sums")
                for c in range(NCH_S):
                    nc.scalar.activation(
                        out=p_sb[:, c * CH : (c + 1) * CH],
                        in_=ps_s[:, c * CH : (c + 1) * CH],
                        func=EXP,
                        bias=negmax,
                        scale=1.0,
                        accum_out=sums[:, c : c + 1],
                    )
                rs = stats.tile([P, 1], F32, tag="rs", name="rs")
                nc.vector.reduce_sum(out=rs, in_=sums, axis=AX)
                rinv = stats.tile([P, 1], F32, tag="rinv", name="rinv")
                nc.vector.reciprocal(out=rinv, in_=rs)
                return p_sb, rinv

            def attend(sq, p_sb, rinv, n_out_chunks=NCH_C):
                pt_ps = ptpsum.tile([P, NT_S, P], BF16, tag="pt", name="ptps")
                for skt in range(NT_S):
                    nc.tensor.transpose(
                        pt_ps[:, skt, :], p_sb[:, skt * P : (skt + 1) * P], eye_h
                    )
                pT = attn.tile([P, NT_S, P], BF16, tag="pT", name="pT")
                nc.vector.tensor_copy(out=pT, in_=pt_ps)
                ps_o = opsum.tile([P, C], F32, tag="o", name="ops")
                for cch in range(NCH_C):
                    sl = ps_o[:, cch * CH : (cch + 1) * CH]
                    for skt in range(NT_S):
                        nc.tensor.matmul(
                            sl,
                            pT[:, skt, :],
                            v_res[skt][:, cch * CH : (cch + 1) * CH],
                            start=(skt == 0),
                            stop=(skt == NT_S - 1),
                        )
                o_sb = attn.tile([P, C], F32, tag="osb", name="osb")
                oc = C // n_out_chunks
                for i in range(n_out_chunks):
                    nc.vector.scalar_tensor_tensor(
                        out=o_sb[:, i * oc : (i + 1) * oc],
                        in0=ps_o[:, i * oc : (i + 1) * oc],
                        scalar=rinv,
                        in1=bv_sb[:, i * oc : (i + 1) * oc],
                        op0=MUL,
                        op1=ADD,
                    )
                    eng = nc.gpsimd if i % 2 == 0 else nc.sync
                    eng.dma_start(
                        out=out[sq * P : (sq + 1) * P, i * oc : (i + 1) * oc],
                        in_=o_sb[:, i * oc : (i + 1) * oc],
                    )

            qts = {0: holders["qt0"], 1: holders["qt1"]}
            prev = None
            for sq in range(NT_S):
                if sq + 2 < NT_S:
                    qts[sq + 2] = load_q(sq + 2)
                st = scores(sq, qts.pop(sq))
                if prev is not None:
                    attend(sq - 1, *prev)
                prev = st
            attend(NT_S - 1, *prev, n_out_chunks=4)


_BUILT = {}


def _build():
    if "nc" not in _BUILT:
        nc = bacc.Bacc(
            "TRN2",
            target_bir_lowering=False,
            debug=False,
            num_devices=B,
        )
        with tile.TileContext(nc) as tc:
            _emit(tc)
        nc.compile()
        _BUILT["nc"] = nc
    return _BUILT["nc"]


def kernel_with_results(trace=False, **inputs):
    import ml_dtypes

    nc = _build()
    eye = np.eye(P, dtype=np.float32)
    bvh = np.asarray(inputs["bv"], dtype=np.float32).astype(ml_dtypes.bfloat16)
    in_maps = []
    for i in range(B):
        in_maps.append(
            {
                "x1": np.ascontiguousarray(inputs["x1"][i], dtype=np.float32),
                "x2": np.ascontiguousarray(inputs["x2"][i], dtype=np.float32),
                "x3": np.ascontiguousarray(inputs["x3"][i], dtype=np.float32),
                "Wq": np.ascontiguousarray(inputs["Wq"], dtype=np.float32),
                "Wk": np.ascontiguousarray(inputs["Wk"], dtype=np.float32),
                "Wv": np.ascontiguousarray(inputs["Wv"], dtype=np.float32),
                "bq": np.ascontiguousarray(inputs["bq"], dtype=np.float32),
                "bk": np.ascontiguousarray(inputs["bk"], dtype=np.float32),
                "bvh": bvh,
                "eye": eye,
            }
        )
    res = run_bass_kernel_spmd(nc, in_maps, core_ids=list(range(B)), trace=trace)
    outs = np.stack([r["out"] for r in res.results], axis=0).astype(np.float32)
    return outs, res


def kernel(**inputs):
    outs, _ = kernel_with_results(trace=False, **inputs)
    return outs
